# revision 1
# baseline (speedup 1.0000x reference)
"""Masked dot-product attention (B=64, L=1024, D=64, fp32) on 8 NeuronCores.

Strategy (data-parallel over batch, per the sharding hint):
  - Batches are sorted by valid_len (descending) and dealt round-robin to the
    8 cores so every core gets one batch from each of 8 "rank groups"; the
    per-slot key-block loop count is baked at build time as the max over that
    slot's rank group.  Key blocks that are entirely masked are never computed.
  - Scores are computed transposed, S^T[k, q] = K @ Q^T, via
    matmul(lhsT=K^T_slice, rhs=Q^T) so that the softmax axis (k) lands on the
    partition dim.  Q and K are passed pre-transposed [D, L] per batch (host
    layout choice at shard time; there is no 4-byte DMA transpose on TRN2).
  - The sequence mask is fused into the exp: ScalarE computes
    P^T = exp(S^T/8 + bias_k) with a per-partition bias column that is 0 for
    valid keys and -1e6 for masked keys (exp underflows to exactly 0).
  - AV uses V augmented with a ones column: O'^T = [V | 1]^T @ P^T, so row 64
    of the accumulator is the softmax denominator for free.
  - Normalization: VectorE reciprocal of the denominator row, replicated
    across partitions by a step-0 free-dim SBUF->SBUF DMA (PE ones-matmul
    broadcast for the final slot, where latency matters), then one
    VectorE multiply.
All matmuls run in float32r (~1.2e-4 relative error, full PE rate).

Scheduling notes (the in-order engine streams make emission order matter):
  - kb loop is software-pipelined: QK(kb+1) is emitted before AV(kb) so PE
    never parks behind an AV that waits on ScalarE's exp.
  - Pair/slot input DMAs are prefetched one slot ahead; the first pair's
    loads are split so the first QK only waits on ~300KB.
  - The divide epilogue is deferred into the next slot's loop and split into
    independent q-halves to shorten the end-of-kernel serial chain.
"""

import math
from contextlib import ExitStack

import numpy as np

import concourse.bass as bass
import concourse.bacc as bacc
import concourse.mybir as mybir
import concourse.tile as tile
from concourse.bass_utils import run_bass_kernel_spmd

F32 = mybir.dt.float32
F32R = mybir.dt.float32r
EXP = mybir.ActivationFunctionType.Exp

B, L, D = 64, 1024, 64
N_CORES = 8
SLOTS = B // N_CORES  # batches per core
KB = 128              # key-block size (partition dim of S^T)
N_KB = L // KB        # max key blocks
QH = 512              # q chunk per matmul (fp32 moving-operand max)
NQH = L // QH
NEG = -1000000.0


def build_kernel(counts):
    """counts[s] = number of 128-wide key blocks to process for slot s."""
    nc = bacc.Bacc()

    qt_d = nc.dram_tensor("qt", [SLOTS, D, L], F32R, kind="ExternalInput")
    kt_d = nc.dram_tensor("kt", [SLOTS, D, L], F32R, kind="ExternalInput")
    v_d = nc.dram_tensor("v", [SLOTS, L, D + 1], F32R, kind="ExternalInput")
    bias_d = nc.dram_tensor("bias", [KB, SLOTS * N_KB], F32, kind="ExternalInput")
    out_d = nc.dram_tensor("out", [SLOTS, D, L], F32, kind="ExternalOutput")

    with tile.TileContext(nc) as tc, ExitStack() as ctx:
        const_pool = ctx.enter_context(tc.tile_pool(name="const", bufs=1))
        qk_pool = ctx.enter_context(tc.tile_pool(name="qk", bufs=3))
        v_pool = ctx.enter_context(tc.tile_pool(name="v", bufs=4))
        p_pool = ctx.enter_context(tc.tile_pool(name="p", bufs=6))
        ep_pool = ctx.enter_context(tc.tile_pool(name="ep", bufs=4))
        out_pool = ctx.enter_context(tc.tile_pool(name="out", bufs=4))
        psum_s = ctx.enter_context(tc.tile_pool(name="psum_s", bufs=2, space="PSUM"))
        psum_o = ctx.enter_context(tc.tile_pool(name="psum_o", bufs=2, space="PSUM"))

        bias_t = const_pool.tile([KB, SLOTS * N_KB], F32)
        warm_t = const_pool.tile([1, 1], F32)
        ones_t = const_pool.tile([1, D], F32R)

        pair_tiles: dict[int, tuple] = {}
        v_tiles: dict[int, object] = {}
        pair_order = [1, 2, 3, 0]  # big pair last: tail epilogues hide in its long loops
        slot_order = [2 * p + h for p in pair_order for h in range(2)]
        next_pair = {pair_order[i]: pair_order[i + 1] for i in range(len(pair_order) - 1)}
        next_slot = {slot_order[i]: slot_order[i + 1] for i in range(len(slot_order) - 1)}

        def load_pair(p):
            if p in pair_tiles:
                return
            n_max = counts[2 * p]
            # Two batches packed on the partition dim: even batch in
            # partitions 0-63, odd batch in 64-127.
            qt_t = qk_pool.tile([2 * D, L], F32R, tag="qt", name="qt_t")
            kt_t = qk_pool.tile([2 * D, L], F32R, tag="kt", name="kt_t")
            src_q = qt_d[2 * p : 2 * p + 2].rearrange("b d l -> (b d) l")
            src_k = kt_d[2 * p : 2 * p + 2].rearrange("b d l -> (b d) l")
            if not pair_tiles:
                # Piecewise: the first slot's kb-0 QKs only wait on the kt
                # head block + their own 64 qt rows (~320KB, 2 gens).
                nc.sync.dma_start(kt_t[:, :KB], src_k[:, :KB])
                nc.sync.dma_start(qt_t[:D, :], src_q[:D, :])
                nc.sync.dma_start(qt_t[D:, :], src_q[D:, :])
                if n_max > 1:
                    nc.sync.dma_start(
                        kt_t[:, KB : n_max * KB], src_k[:, KB : n_max * KB]
                    )
            else:
                nc.sync.dma_start(qt_t[:], src_q)
                nc.sync.dma_start(kt_t[:, : n_max * KB], src_k[:, : n_max * KB])
            pair_tiles[p] = (qt_t, kt_t)

        def load_v(s):
            if s in v_tiles:
                return
            n_kb = counts[s]
            v_t = v_pool.tile([KB, N_KB, D + 1], F32R, name="v_t")
            nc.gpsimd.dma_start(
                v_t[:, :n_kb, :],
                v_d[s].rearrange("(n p) d -> p n d", p=KB)[:, :n_kb, :],
            )
            v_tiles[s] = v_t

        def qk(s_ps, rows, kt_t, qt_t, kb):
            for qh in range(NQH):
                nc.tensor.matmul(
                    s_ps[:, qh * QH : (qh + 1) * QH],
                    kt_t[rows, kb * KB : (kb + 1) * KB],
                    qt_t[rows, qh * QH : (qh + 1) * QH],
                    start=True,
                    stop=True,
                )

        def make_tail(s, o_ps, rec_b, qh):
            # Deferred epilogue part B for one q-half: divide and store.
            c0, c1 = qh * QH, (qh + 1) * QH

            last = s == slot_order[-1]

            def tail():
                out_sb = out_pool.tile([D, QH], F32, name="out_sb")
                nch = 1
                cw = QH // nch
                for ch in range(nch):
                    nc.vector.tensor_tensor(
                        out_sb[:, ch * cw : (ch + 1) * cw],
                        o_ps[:D, c0 + ch * cw : c0 + (ch + 1) * cw],
                        rec_b[:, c0 + ch * cw : c0 + (ch + 1) * cw],
                        op=mybir.AluOpType.mult,
                    )
                    nc.sync.dma_start(
                        out_d[s][:, c0 + ch * cw : c0 + (ch + 1) * cw],
                        out_sb[:, ch * cw : (ch + 1) * cw],
                    )

            return tail

        load_pair(pair_order[0])
        # bias rides the SWDGE path so the first exp isn't queued behind
        # the HWDGE input loads.
        nc.gpsimd.dma_start(bias_t[:], bias_d[:])
        # Warm the exp table set while the first DMAs run; also build a
        # ones row (exp of 0 * bias) for the tail's PE broadcast.
        nc.scalar.activation(warm_t[:], bias_t[0:1, 0:1], EXP)
        nc.scalar.activation(ones_t[:], bias_t[0:1, :D], EXP, scale=0.0)
        load_v(slot_order[0])

        # Flat (slot, kb) work list, software-pipelined at depth 2 across
        # slot boundaries: the PE stream is QK(i+1), AV(i-1), so PE never
        # refills the pipeline at a slot change and AV only ever consumes
        # an exp that finished a full iteration ago.
        work = [(s, kb) for s in slot_order for kb in range(counts[s])]
        n_work = len(work)
        slot_first = {s: i for i, (s, kb) in reversed(list(enumerate(work)))}
        o_tiles: dict[int, object] = {}
        s_tiles: dict[tuple, object] = {}
        p_tiles: dict[tuple, object] = {}
        pending_tails: list = []
        tail_due: int = -1

        def emit_qk(i):
            s, kb = work[i]
            pair, half = divmod(s, 2)
            if kb == 0:
                # Slot prologue: prefetch upcoming inputs.
                nxt = slot_order.index(s) + 1
                if nxt < SLOTS:
                    load_v(slot_order[nxt])
                    if nxt + 1 < SLOTS:
                        load_v(slot_order[nxt + 1])
                if half == 0 and pair in next_pair:
                    load_pair(next_pair[pair])
                if half == 1 and pair in next_pair and next_pair[pair] in next_pair:
                    load_pair(next_pair[next_pair[pair]])
            qt_t, kt_t = pair_tiles[pair]
            rows = slice(D * half, D * half + D)
            s_tiles[(s, kb)] = psum_s.tile([KB, L], F32, tag="s", name="s_ps")
            qk(s_tiles[(s, kb)], rows, kt_t, qt_t, kb)

        def emit_av(i):
            s, kb = work[i]
            n_kb = counts[s]
            if kb == 0:
                o_tiles[s] = psum_o.tile([D + 1, L], F32, tag="o", name="o_ps")
            o_ps = o_tiles[s]
            p_t = p_tiles.pop((s, kb))
            for qh in range(NQH):
                nc.tensor.matmul(
                    o_ps[:, qh * QH : (qh + 1) * QH],
                    v_tiles[s][:, kb, :],
                    p_t[:, qh * QH : (qh + 1) * QH],
                    start=(kb == 0),
                    stop=(kb == n_kb - 1),
                )
            if kb == n_kb - 1:
                emit_epilogue_a(s)

        def emit_epilogue_a(s):
            # Reciprocal of the denominator row, then partition-replicate.
            nonlocal pending_tails, tail_due
            if pending_tails:
                for t in pending_tails:
                    t()
                pending_tails = []
            o_ps = o_tiles[s]
            last = s == slot_order[-1]
            rdt = F32R if last else F32
            rec_row = ep_pool.tile([1, L], rdt, tag="l", name="rec_row")
            rec_b = ep_pool.tile([D, L], rdt, tag="rec", name="rec_b")
            if not last:
                nc.vector.reciprocal(rec_row[:], o_ps[D : D + 1, :])
                row_ap = rec_row[:]
                bcast_src = bass.AP(
                    row_ap.tensor, row_ap.offset,
                    [list(row_ap.ap)[0], [0, D]] + list(row_ap.ap)[1:],
                )
                nc.gpsimd.dma_start(rec_b[:], bcast_src)
            else:
              for qh in range(NQH):
                c0, c1 = qh * QH, (qh + 1) * QH
                with nc.allow_low_precision("f32r label for PE-broadcast tail"):
                    nc.vector.reciprocal(rec_row[:, c0:c1], o_ps[D : D + 1, c0:c1])
                if last:
                    # Tail: PE broadcast + ScalarE copy (both idle by now;
                    # keeps the serial DVE chain to recip + multiply).
                    bc_ps = psum_s.tile([D, QH], F32, tag="s", name="bc_ps")
                    nc.tensor.matmul(
                        bc_ps[:], ones_t[:], rec_row[:, c0:c1],
                        start=True, stop=True,
                    )
                    nc.scalar.copy(rec_b[:, c0:c1], bc_ps[:])
                else:
                    row_ap = rec_row[:, c0:c1]
                    bcast_src = bass.AP(
                        row_ap.tensor, row_ap.offset,
                        [list(row_ap.ap)[0], [0, D]] + list(row_ap.ap)[1:],
                    )
                    nc.gpsimd.dma_start(rec_b[:, c0:c1], bcast_src)
            pending_tails = [make_tail(s, o_ps, rec_b, qh) for qh in range(NQH)]
            tail_due = min(slot_first.get(slot_order[slot_order.index(s) + 1], 0) + 3
                           if slot_order.index(s) + 1 < SLOTS else 0, n_work - 1)

        emit_qk(0)
        for i in range(n_work):
            if i + 1 < n_work:
                emit_qk(i + 1)
            if pending_tails and i >= tail_due:
                for t in pending_tails:
                    t()
                pending_tails = []
            s, kb = work[i]
            p_tiles[(s, kb)] = p_pool.tile([KB, L], F32R, name="p_t")
            nc.scalar.activation(
                p_tiles[(s, kb)][:],
                s_tiles.pop((s, kb))[:],
                EXP,
                bias=bias_t[:, s * N_KB + kb : s * N_KB + kb + 1],
                scale=1.0 / math.sqrt(D),
            )
            if i >= 1:
                emit_av(i - 1)
        emit_av(n_work - 1)
        for t in pending_tails:
            t()

    nc.finalize()
    return nc


_NC_CACHE: dict[tuple, object] = {}


def _prepare(queries, keys, values, valid_lens):
    queries = np.ascontiguousarray(queries, dtype=np.float32)
    keys = np.ascontiguousarray(keys, dtype=np.float32)
    values = np.ascontiguousarray(values, dtype=np.float32)
    valid_lens = np.asarray(valid_lens)
    assert queries.shape == (B, L, D), queries.shape
    vl = valid_lens.astype(np.int64)

    # Sort batches by valid_len descending; slot s on core c gets the batch
    # of rank s*8 + c.  Each slot's loop count covers the max valid_len in
    # its rank group, so one instruction stream fits all cores.
    order = np.argsort(-vl, kind="stable")
    counts = tuple(
        max(1, math.ceil(int(vl[order[s * N_CORES]]) / KB)) for s in range(SLOTS)
    )
    # Pairs share a K^T tile sized by the even slot; counts are descending.
    nc = _NC_CACHE.get(counts)
    if nc is None:
        nc = build_kernel(counts)
        _NC_CACHE[counts] = nc

    col = np.arange(L)
    in_maps = []
    for c in range(N_CORES):
        batch_idx = [int(order[s * N_CORES + c]) for s in range(SLOTS)]
        qt = np.ascontiguousarray(
            queries[batch_idx].transpose(0, 2, 1)
        )  # [SLOTS, D, L]
        kt = np.ascontiguousarray(keys[batch_idx].transpose(0, 2, 1))
        v = np.concatenate(
            [values[batch_idx], np.ones((SLOTS, L, 1), np.float32)], axis=2
        )
        bias = np.zeros((KB, SLOTS * N_KB), dtype=np.float32)
        for s in range(SLOTS):
            mask = (col >= vl[batch_idx[s]]).astype(np.float32) * NEG  # [L]
            bias[:, s * N_KB : (s + 1) * N_KB] = mask.reshape(N_KB, KB).T
        in_maps.append({"qt": qt, "kt": kt, "v": v, "bias": bias})
    return nc, in_maps, order


def _unshard(res, order):
    out = np.empty((B, L, D), dtype=np.float32)
    for c in range(N_CORES):
        o = res.results[c]["out"]  # [SLOTS, D, L]
        for s in range(SLOTS):
            out[int(order[s * N_CORES + c])] = o[s].T
    return out


def kernel(queries, keys, values, valid_lens):
    nc, in_maps, order = _prepare(queries, keys, values, valid_lens)
    res = run_bass_kernel_spmd(nc, in_maps, core_ids=list(range(N_CORES)))
    return _unshard(res, order)


def trace_run(queries, keys, values, valid_lens):
    """Like kernel() but traced; returns BassKernelResults (for test.py)."""
    nc, in_maps, order = _prepare(queries, keys, values, valid_lens)
    res = run_bass_kernel_spmd(
        nc, in_maps, core_ids=list(range(N_CORES)), trace=True
    )
    res.full_output = _unshard(res, order)
    return res



# revision 4
# speedup vs baseline: 1.2028x; 1.2028x over previous
"""Masked dot-product attention (B=64, L=1024, D=64, fp32) on 8 NeuronCores.

Strategy (data-parallel over batch, per the sharding hint):
  - Batches are sorted by valid_len (descending) and dealt round-robin to the
    8 cores; slot s's key-block loop count is the max over its rank group, so
    one SPMD instruction stream fits all cores and fully-masked key blocks
    are never computed.
  - The sequence mask rides INSIDE the QK matmul as an extra contraction row:
    K^T is augmented with a mask row (0 / -60000 per key) and Q^T with a ones
    row, so S' = K^T.T @ Q^T + m[k] needs no per-block exp bias.  That makes
    the exp a pure elementwise op over PSUM columns, so one ScalarE
    instruction spans THREE 512-column score units regardless of which key
    block they belong to (amortizes the ~185ns per-instruction SBUF access
    latency; ScalarE is the bottleneck engine at ~1 elem/cycle/partition).
  - Scores are computed transposed, S^T[k, q], 512 q at a time:
    matmul(lhsT=K^T_aug[65, 128], rhs=Q^T_aug[65, 512]) -> PSUM [128, 512].
    Work is a flat stream of (slot, kb, qh) units; exp groups of 3 units
    live in [128, 1536] PSUM tiles (3 banks, double-buffered = 6 banks).
  - P = exp(S'/8) is written as float16 (rel err ~5e-4, well inside the
    2e-2 gate).  AV is P-chunk-stationary: for each 128-query block,
    matmul(lhsT=P^T[128k, 128q], rhs=V_aug[128k, 65]) accumulates
    O[q, d] over key blocks in PSUM.  LdWeights is free on the PE, and the
    65 fp16 moving rows cost 65 cycles, so AV is ~2x cheaper than the
    moving-P orientation and the output lands Q-MAJOR.
  - V_aug has a ones column, so O[:, 64] is the softmax denominator.
    Normalization collapses to a [128,1]-per-partition scalar multiply:
    one DVE reciprocal on the 4 denominator columns + one tensor_tensor
    with a stride-0 broadcast AP.  No cross-partition broadcast of any
    kind (the baseline's PE/DMA reciprocal-row machinery is gone).
  - Outputs are written [q, d] per slot — the natural layout — so the
    host-side unshard is a pure batch reorder.

Engine budget per core (cost model): ScalarE ~42us (saturated), PE ~28us,
DVE ~8us, Pool: v-loads only.  Inputs fp16 (Q/K host-converted; scores err
~4e-3 absolute pre-softmax-scale), matmul accumulation in PSUM f32.
"""

import math
from contextlib import ExitStack

import numpy as np

import concourse.bass as bass
import concourse.bacc as bacc
import concourse.mybir as mybir
import concourse.tile as tile
from concourse.bass_utils import run_bass_kernel_spmd

F32 = mybir.dt.float32
F16 = mybir.dt.float16
EXP = mybir.ActivationFunctionType.Exp

B, L, D = 64, 1024, 64
N_CORES = 8
SLOTS = B // N_CORES  # batches per core
KB = 128              # key-block size (partition dim of S^T)
N_KB = L // KB
QH = 512              # q columns per matmul unit (moving-operand max)
NQH = L // QH
GROUP = 3             # 512-col score units per exp instruction (3 PSUM banks)
MASK_VAL = -60000.0   # fits fp16; exp(-60000/8) == 0
DV = D + 1            # V columns + ones (denominator) column


def build_kernel(counts):
    """counts[s] = number of 128-wide key blocks to process for slot s."""
    nc = bacc.Bacc()

    qt_d = nc.dram_tensor("qt", [SLOTS, DV, L], F16, kind="ExternalInput")
    kt_d = nc.dram_tensor("kt", [SLOTS, DV, L], F16, kind="ExternalInput")
    v_d = nc.dram_tensor("v", [SLOTS, KB, N_KB, DV], F16, kind="ExternalInput")
    out_d = nc.dram_tensor("out", [SLOTS, L, D], F32, kind="ExternalOutput")

    # Flat unit stream, qh-major inside each slot so the first exp only
    # needs half of qt; kb ascending keeps PSUM accumulation ordered.
    units = [
        (s, kb, qh)
        for s in range(SLOTS)
        for qh in range(NQH)
        for kb in range(counts[s])
    ]
    # First groups are small so ScalarE starts as early as possible.
    lead = [1, 2]
    groups = []
    pos = 0
    for n in lead:
        if pos < len(units):
            groups.append(units[pos : pos + n])
            pos += n
    while pos < len(units):
        groups.append(units[pos : pos + GROUP])
        pos += GROUP
    n_groups = len(groups)

    with tile.TileContext(nc) as tc, ExitStack() as ctx:
        const_pool = ctx.enter_context(tc.tile_pool(name="const", bufs=1))
        qk_pool = ctx.enter_context(tc.tile_pool(name="qk", bufs=3))
        v_pool = ctx.enter_context(tc.tile_pool(name="v", bufs=3))
        p_pool = ctx.enter_context(tc.tile_pool(name="p", bufs=4))
        ep_pool = ctx.enter_context(tc.tile_pool(name="ep", bufs=4))
        out_pool = ctx.enter_context(tc.tile_pool(name="out", bufs=4))
        psum_s = ctx.enter_context(tc.tile_pool(name="psum_s", bufs=2, space="PSUM"))
        psum_o = ctx.enter_context(tc.tile_pool(name="psum_o", bufs=1, space="PSUM"))

        qt_tiles: dict[int, object] = {}
        kt_tiles: dict[int, object] = {}
        v_tiles: dict[int, object] = {}
        o_tiles: dict[tuple, object] = {}
        s_tiles: dict[int, object] = {}
        p_tiles: dict[int, object] = {}

        def load_slot(s):
            if s >= SLOTS or s in qt_tiles:
                return
            n_kb = counts[s]
            qt_t = qk_pool.tile([DV, L], F16, tag="qt", name="qt_t")
            kt_t = qk_pool.tile([DV, L], F16, tag="kt", name="kt_t")
            if s == 0:
                # Piecewise so the first QK only waits on ~1.2KB/partition.
                head = min(GROUP, n_kb) * KB
                nc.sync.dma_start(kt_t[:, :head], kt_d[0][:, :head])
                nc.scalar.dma_start(qt_t[:, :QH], qt_d[0][:, :QH])
                nc.scalar.dma_start(qt_t[:, QH:], qt_d[0][:, QH:])
                if n_kb * KB > head:
                    nc.sync.dma_start(
                        kt_t[:, head : n_kb * KB], kt_d[0][:, head : n_kb * KB]
                    )
            else:
                nc.scalar.dma_start(qt_t[:], qt_d[s])
                nc.sync.dma_start(kt_t[:, : n_kb * KB], kt_d[s][:, : n_kb * KB])
            qt_tiles[s] = qt_t
            kt_tiles[s] = kt_t

        def load_v(s):
            if s >= SLOTS or s in v_tiles:
                return
            n_kb = counts[s]
            v_t = v_pool.tile([KB, N_KB, DV], F16, name="v_t")
            nc.gpsimd.dma_start(v_t[:, :n_kb, :], v_d[s][:, :n_kb, :])
            v_tiles[s] = v_t

        def emit_qk(i):
            st = psum_s.tile([KB, GROUP * QH], F32, name="s_ps")
            s_tiles[i] = st
            for u, (s, kb, qh) in enumerate(groups[i]):
                if kb == 0 and qh == 0:
                    load_slot(s + 1)
                    load_v(s + 1)
                nc.tensor.matmul(
                    st[:, u * QH : (u + 1) * QH],
                    kt_tiles[s][:, kb * KB : (kb + 1) * KB],
                    qt_tiles[s][:, qh * QH : (qh + 1) * QH],
                    start=True,
                    stop=True,
                )

        def emit_exp(i):
            w = len(groups[i]) * QH
            pt = p_pool.tile([KB, GROUP * QH], F16, name="p_t")
            p_tiles[i] = pt
            nc.scalar.activation(
                pt[:, :w], s_tiles.pop(i)[:, :w], EXP, scale=1.0 / math.sqrt(D)
            )

        def emit_av(i):
            pt = p_tiles.pop(i)
            for u, (s, kb, qh) in enumerate(groups[i]):
                tag = "oA" if qh == 0 else "oB"
                if kb == 0:
                    o_tiles[(s, qh)] = psum_o.tile(
                        [KB, NQH * 2 * DV], F32, tag=tag, name=tag
                    )
                o = o_tiles[(s, qh)]
                last = kb == counts[s] - 1
                for j in range(4):
                    # start=True zeroes the WHOLE PSUM bank, so only the
                    # very first matmul into this o tile may use it; the
                    # other three q-block regions accumulate onto the
                    # zeroed bank.
                    nc.tensor.matmul(
                        o[:, j * DV : (j + 1) * DV],
                        pt[:, u * QH + j * KB : u * QH + (j + 1) * KB],
                        v_tiles[s][:, kb, :],
                        start=(kb == 0 and j == 0),
                        stop=last,
                        skip_group_check=True,
                    )
                if last:
                    emit_epilogue(s, qh)
                    if qh == NQH - 1:
                        qt_tiles.pop(s)
                        kt_tiles.pop(s)
                        v_tiles.pop(s)

        def emit_epilogue(s, half):
            o = o_tiles.pop((s, half))
            rec = ep_pool.tile([KB, 4], F32, name="rec")
            nc.vector.reciprocal(rec[:], o[:, D::DV])
            osb = out_pool.tile([KB, 4 * D], F32, name="osb")
            nc.vector.tensor_tensor(
                osb[:].rearrange("p (a b) -> p a b", b=D),
                o[:].rearrange("p (a b) -> p a b", b=DV)[:, :, :D],
                rec[:].rearrange("p (a b) -> p a b", b=1).broadcast_to([KB, 4, D]),
                op=mybir.AluOpType.mult,
            )
            dst = out_d[s][half * QH : (half + 1) * QH].rearrange(
                "(j p) d -> p j d", p=KB
            )
            nc.sync.dma_start(dst, osb[:].rearrange("p (a b) -> p a b", b=D))

        # Prologue: warm the exp table off the critical path, start loads.
        warm_in = const_pool.tile([1, 1], F32)
        warm_out = const_pool.tile([1, 1], F32)
        nc.gpsimd.memset(warm_in[:], 0.0)
        nc.scalar.activation(warm_out[:], warm_in[:], EXP)
        load_slot(0)
        load_v(0)

        emit_qk(0)
        for i in range(n_groups):
            if i + 1 < n_groups:
                emit_qk(i + 1)
            emit_exp(i)
            if i >= 1:
                emit_av(i - 1)
        emit_av(n_groups - 1)

    nc.finalize()
    return nc


_NC_CACHE: dict[tuple, object] = {}


def _prepare(queries, keys, values, valid_lens):
    queries = np.ascontiguousarray(queries, dtype=np.float32)
    keys = np.ascontiguousarray(keys, dtype=np.float32)
    values = np.ascontiguousarray(values, dtype=np.float32)
    valid_lens = np.asarray(valid_lens)
    assert queries.shape == (B, L, D), queries.shape
    vl = valid_lens.astype(np.int64)

    # Sort batches by valid_len descending; slot s on core c gets the batch
    # of rank s*8 + c.  counts[s] covers the rank-group max, so one SPMD
    # instruction stream fits all cores.
    order = np.argsort(-vl, kind="stable")
    counts = tuple(
        max(1, math.ceil(int(vl[order[s * N_CORES]]) / KB)) for s in range(SLOTS)
    )
    nc = _NC_CACHE.get(counts)
    if nc is None:
        nc = build_kernel(counts)
        _NC_CACHE[counts] = nc

    col = np.arange(L)
    in_maps = []
    for c in range(N_CORES):
        batch_idx = [int(order[s * N_CORES + c]) for s in range(SLOTS)]
        # Q^T / K^T with the extra contraction row: ones for Q, mask for K.
        qt = np.empty((SLOTS, DV, L), np.float16)
        qt[:, :D, :] = queries[batch_idx].transpose(0, 2, 1)
        qt[:, D, :] = 1.0
        kt = np.empty((SLOTS, DV, L), np.float16)
        kt[:, :D, :] = keys[batch_idx].transpose(0, 2, 1)
        kt[:, D, :] = (col[None, :] >= vl[batch_idx, None]) * np.float16(MASK_VAL)
        # V with ones column, pre-tiled [KB, N_KB, DV] per slot.
        v = np.empty((SLOTS, L, DV), np.float16)
        v[:, :, :D] = values[batch_idx]
        v[:, :, D] = 1.0
        v = np.ascontiguousarray(
            v.reshape(SLOTS, N_KB, KB, DV).transpose(0, 2, 1, 3)
        )
        in_maps.append({"qt": qt, "kt": kt, "v": v})
    return nc, in_maps, order


def _unshard(res, order):
    out = np.empty((B, L, D), dtype=np.float32)
    for c in range(N_CORES):
        o = res.results[c]["out"]  # [SLOTS, L, D]
        for s in range(SLOTS):
            out[int(order[s * N_CORES + c])] = o[s]
    return out


def kernel(queries, keys, values, valid_lens):
    nc, in_maps, order = _prepare(queries, keys, values, valid_lens)
    res = run_bass_kernel_spmd(nc, in_maps, core_ids=list(range(N_CORES)))
    return _unshard(res, order)


def trace_run(queries, keys, values, valid_lens):
    """Like kernel() but traced; returns BassKernelResults (for test.py)."""
    nc, in_maps, order = _prepare(queries, keys, values, valid_lens)
    res = run_bass_kernel_spmd(
        nc, in_maps, core_ids=list(range(N_CORES)), trace=True
    )
    res.full_output = _unshard(res, order)
    return res
